# revision 49
# baseline (speedup 1.0000x reference)
"""Trainium2 Bass kernel for causal degree-2 polynomial attention.

The reference module is chunked linear attention with kernel weight
(q.k)^2, which is mathematically exact causal polynomial attention:

    out_q = sum_{k<=q} (Q_q.K_k)^2 V_k / (EPS + sum_{k<=q} (Q_q.K_k)^2)

Sharding: 16 (batch, head) pairs across 8 cores -> 2 pairs/core, fully
data-parallel (matches the chunk-local-cumsum hint; no collectives).

Per-core device algorithm (the two pairs are emitted interleaved so
their independent dependency chains fill each other's pipeline stalls):
  - load Q,K,V [2048, 64] fp32 on both HWDGE queues, cast to bf16 on
    DVE in 256-col chunks, PE-transpose Q,K (pair-stacked) into bf16
    [d, t] layouts; Q duplicated onto both partition halves with
    per-block SBUF->SBUF DMAs so D'[k,q] = K Q^T runs as two row-tiled
    K=64 bf16 matmuls per slot with N=512 query blocks.
  - square D' (fp32 PSUM -> bf16 SBUF on ACT, every 5th on DVE via
    cast-copy + multiply), causal-mask diagonal 128-col windows with
    gpsimd affine_select, zero-fill fully masked columns.
  - accumulate [V|1]^T C' into PSUM [65, 512] (bf16 x bf16 -> fp32).
  - PE-transpose back to [q, 65], reciprocal(Z+eps), scale, store each
    query block as soon as it finishes.
"""

import os
import sys

for _p in ("/root/.axon_site", "/root/.axon_site/_ro/trn_rl_repo",
           "/root/.axon_site/_ro/pypackages", "/opt/trn_rl_repo", "/opt/pypackages"):
    if os.path.isdir(_p) and _p not in sys.path:
        sys.path.append(_p)

import numpy as np

import concourse.bacc as bacc
import concourse.mybir as mybir
import concourse.tile as tile
from concourse.bass_utils import run_bass_kernel_spmd

F32 = mybir.dt.float32
BF16 = mybir.dt.bfloat16
EPS = 1e-5

N_CORES = 8
T = 2048          # tokens
D = 64            # head dim
PAIRS = 2         # (b, h) pairs per core
NKB = T // 128    # 16 key blocks of 128
QB = 512          # query block width
NQB = T // QB     # 4 query blocks

_CACHE = {}


def _interleave(lists):
    out = []
    n = max(len(l) for l in lists)
    for j in range(n):
        for l in lists:
            if j < len(l):
                out.append(l[j])
    return out


def build_nc(mmdt=BF16, dve_every=5):
    nc = bacc.Bacc("TRN2", target_bir_lowering=False, debug=False)
    MMDT = mmdt

    ins = {}
    outs = {}
    for p in range(PAIRS):
        for nm in ("q", "k", "v"):
            ins[f"{nm}{p}"] = nc.dram_tensor(f"{nm}{p}", [T, D], F32, kind="ExternalInput").ap()
        outs[p] = nc.dram_tensor(f"o{p}", [T, D], F32, kind="ExternalOutput").ap()
    ident = nc.dram_tensor("ident", [128, 128], F32, kind="ExternalInput").ap()
    ones16 = nc.dram_tensor("ones16", [128, NKB], F32, kind="ExternalInput").ap()

    with tile.TileContext(nc) as tc:
        with (
            tc.tile_pool(name="const", bufs=1) as cpool,
            tc.tile_pool(name="persist", bufs=1) as perpool,
            tc.tile_pool(name="cprime", bufs=8) as cppool,
            tc.tile_pool(name="small", bufs=6) as smpool,
        ):
            ident_sb = cpool.tile([128, 128], F32, name="ident_sb")
            nc.sync.dma_start(ident_sb[:], ident[:])
            identm_sb = cpool.tile([128, 128], MMDT, name="identm_sb")
            nc.vector.tensor_copy(identm_sb[:], ident_sb[:])

            # ---- persistent per-pair SBUF tensors ----
            qT2 = []   # [128, 2048] mmdt, both halves hold Q^T (duplicated)
            kT2 = []   # [128, 1024] mmdt, top: even key blocks, bottom: odd
            v1 = []    # [128, 16*65] mmdt, col 64 of each 65-group is 1.0
            obuf = []  # [128, 16*64] f32 staged output
            for p in range(PAIRS):
                qT2.append(perpool.tile([128, T], MMDT, name=f"qT2_{p}"))
                kT2.append(perpool.tile([128, T // 2], MMDT, name=f"kT2_{p}"))
                v1.append(perpool.tile([128, NKB * 65], MMDT, name=f"v1_{p}"))
                obuf.append(perpool.tile([128, NKB * 64], F32, name=f"obuf_{p}"))

            # ---- stage A: loads, bf16 casts, PE transposes ----
            with (
                tc.tile_pool(name="stage", bufs=2) as stpool,
                tc.tile_pool(name="pstA", bufs=3, space="PSUM") as pstA,
            ):
                nat = {}
                units_a = [[] for _ in range(PAIRS)]
                for p in range(PAIRS):
                    u = units_a[p]

                    def loads(p=p):
                        qn = stpool.tile([128, T // 2], F32, name=f"qn_{p}", tag=f"natq{p}")
                        kn = stpool.tile([128, T // 2], F32, name=f"kn_{p}", tag=f"natk{p}")
                        vn = stpool.tile([128, T // 2], F32, name=f"vn_{p}", tag=f"natv{p}")
                        on = stpool.tile([128, NKB], F32, name=f"on_{p}", tag="ones")
                        dmae = nc.sync if p == 0 else nc.scalar
                        for tile_, nm in ((qn, "q"), (kn, "k"), (vn, "v")):
                            dmae.dma_start(
                                tile_[:].rearrange("p (n d) -> p n d", n=NKB),
                                ins[f"{nm}{p}"].rearrange("(n p) d -> p n d", p=128),
                            )
                        dmae.dma_start(on[:], ones16[:])
                        # per-pair engine split so both ACT and DVE chew the
                        # stage-A elementwise ramp in parallel
                        cp = (lambda d, s: nc.vector.tensor_copy(d, s)) if p == 0 \
                            else (lambda d, s: nc.scalar.copy(d, s))
                        v1g = v1[p][:].rearrange("p (n c) -> p n c", c=65)
                        cp(v1g[:, :, 64], on[:])
                        cp(v1g[:, :, 0:64], vn[:].rearrange("p (n d) -> p n d", n=NKB))
                        qnb = stpool.tile([128, T // 2], MMDT, name=f"qnb_{p}", tag=f"natqb{p}")
                        knb = stpool.tile([128, T // 2], MMDT, name=f"knb_{p}", tag=f"natkb{p}")
                        # chunked casts so transposes can start early
                        for cki in range(4):
                            ck = slice(cki * 256, (cki + 1) * 256)
                            cp(qnb[:, ck], qn[:, ck])
                            cp(knb[:, ck], kn[:, ck])
                        nat[p] = (qnb, knb)

                    u.append(loads)
                    for a in range(NKB // 2):
                        def transp(p=p, a=a):
                            qnb, knb = nat[p]
                            # two adjacent token blocks transposed at once:
                            # rows 0:64 = block 2a ^T, 64:128 = block 2a+1 ^T
                            tq = pstA.tile([128, 128], MMDT, name="tq", tag="tp")
                            nc.tensor.transpose(tq[:], qnb[:, a * 128:(a + 1) * 128], identm_sb[:])
                            tk = pstA.tile([128, 128], MMDT, name="tk", tag="tp")
                            nc.tensor.transpose(tk[:], knb[:, a * 128:(a + 1) * 128], identm_sb[:])
                            cp = (lambda d, s: nc.vector.tensor_copy(d, s)) if p == 0 \
                                else (lambda d, s: nc.scalar.copy(d, s))
                            cp(kT2[p][:, a * 128:(a + 1) * 128], tk[:])
                            c0 = slice(256 * a, 256 * a + 128)
                            c1 = slice(256 * a + 128, 256 * a + 256)
                            cp(qT2[p][0:64, c0], tq[0:64, :])
                            cp(qT2[p][0:64, c1], tq[64:128, :])
                            if a % 2 == 1:
                                # duplicate finished q-block cols onto bottom half
                                i = a // 2
                                cols = slice(i * QB, (i + 1) * QB)
                                nc.sync.dma_start(qT2[p][64:128, cols], qT2[p][0:64, cols])

                        u.append(transp)

                for f in _interleave(units_a):
                    f()

            # ---- stage B: main blocked attention ----
            with (
                tc.tile_pool(name="psd", bufs=6, space="PSUM") as psd,
                tc.tile_pool(name="psyz", bufs=2, space="PSUM") as psyz,
            ):
                tails = {}
                state = {"n": 0}

                def square(dst, srcap, cols):
                    # ACT squares straight from PSUM; every dve_every-th
                    # block goes to DVE as cast-copy + SBUF multiply.
                    state["n"] += 1
                    if state["n"] % dve_every == 0:
                        tmp = cppool.tile([128, cols], MMDT, name="sqtmp", tag="sqtmp")
                        nc.vector.tensor_copy(tmp[:], srcap)
                        nc.vector.tensor_mul(dst, tmp[:], tmp[:])
                    else:
                        nc.scalar.square(dst, srcap)

                def emit_tail(p, i):
                    """finish query block i of pair p: psumYZ -> output rows."""
                    yz = smpool.tile([65, QB], F32, name="yz", tag="yz")
                    nc.vector.tensor_copy(yz[:], tails[(p, i)][:])
                    for half in range(QB // 128):
                        blk = (QB // 128) * i + half
                        pt = psd.tile([128, 65], F32, name="pt", tag="psd")
                        nc.tensor.transpose(
                            pt[:], yz[:, half * 128:(half + 1) * 128],
                            ident_sb[0:65, 0:65],
                        )
                        zs = smpool.tile([128, 1], F32, name="zs", tag="zs")
                        nc.vector.tensor_scalar_add(zs[:], pt[:, 64:65], EPS)
                        rz = smpool.tile([128, 1], F32, name="rz", tag="rz")
                        nc.vector.reciprocal(rz[:], zs[:])
                        nc.vector.tensor_scalar_mul(
                            obuf[p][:, blk * 64:(blk + 1) * 64], pt[:, 0:64], rz[:]
                        )
                    rows = slice(i * QB, (i + 1) * QB)
                    nc.sync.dma_start(
                        outs[p][rows, :].rearrange("(n p) d -> p n d", p=128),
                        obuf[p][:, i * (QB // 128) * 64:(i + 1) * (QB // 128) * 64]
                            .rearrange("p (n d) -> p n d", d=64),
                    )

                def emit_dpair(p, i, a):
                    """D'[k, q] for key blocks (2a, 2a+1) vs query block i."""
                    qcols = slice(i * QB, (i + 1) * QB)
                    kcols = slice(a * 128, (a + 1) * 128)
                    psA = psd.tile([128, QB], F32, name="psA", tag="psd")
                    psB = psd.tile([128, QB], F32, name="psB", tag="psd")
                    nc.tensor.matmul(
                        psA[:], kT2[p][0:64, kcols], qT2[p][0:64, qcols],
                        start=True, stop=True, tile_position=(0, 0),
                        skip_group_check=True,
                    )
                    nc.tensor.matmul(
                        psB[:], kT2[p][64:128, kcols], qT2[p][64:128, qcols],
                        start=True, stop=True, tile_position=(64, 0),
                        skip_group_check=True,
                    )
                    c2 = cppool.tile([128, 2 * QB], MMDT, name="c2", tag="cp")
                    cE = c2[:, 0:QB]
                    cO = c2[:, QB:2 * QB]
                    for par, (ps, cX) in enumerate(((psA, cE), (psB, cO))):
                        r = 2 * a + par - (QB // 128) * i  # diag sub-block
                        if r < 0:
                            square(cX, ps[:], QB)
                            continue
                        w0 = 128 * r
                        if w0 > 0:
                            nc.gpsimd.memset(cX[:, 0:w0], 0.0)
                        square(cX[:, w0:QB], ps[:, w0:QB], QB - w0)
                        # causal mask on the 128-col diagonal window
                        # (keep where q - k >= 0; iota = f + cm*p + base)
                        nc.gpsimd.affine_select(
                            cX[:, w0:w0 + 128], cX[:, w0:w0 + 128],
                            pattern=[[1, 128]],
                            compare_op=mybir.AluOpType.is_ge, fill=0.0,
                            base=0, channel_multiplier=-1,
                        )
                    return cE, cO

                def emit_cv(p, i, a, cE, cO):
                    yzp = tails[(p, i)]
                    vE = v1[p][:, (2 * a) * 65:(2 * a + 1) * 65]
                    vO = v1[p][:, (2 * a + 1) * 65:(2 * a + 2) * 65]
                    nc.tensor.matmul(
                        yzp[:], vE, cE,
                        start=(a == 0), stop=False, skip_group_check=True,
                    )
                    nc.tensor.matmul(
                        yzp[:], vO, cO,
                        start=False, stop=(a == 2 * i + 1), skip_group_check=True,
                    )

                pend = {}
                units_b = [[] for _ in range(PAIRS)]
                for p in range(PAIRS):
                    u = units_b[p]
                    for i in range(NQB):
                        for a in range(2 * i + 2):
                            def step(p=p, i=i, a=a):
                                if (i, a) == (0, 0):
                                    pend[p] = emit_dpair(p, 0, 0)
                                cur = pend[p]
                                # software pipeline: next D' ahead of this CV
                                if a < 2 * i + 1:
                                    pend[p] = emit_dpair(p, i, a + 1)
                                elif i + 1 < NQB:
                                    pend[p] = emit_dpair(p, i + 1, 0)
                                if a == 0:
                                    tails[(p, i)] = psyz.tile(
                                        [65, QB], F32, name=f"yzp_{p}_{i}", tag="yzp"
                                    )
                                emit_cv(p, i, a, *cur)
                                if a == 2 * i + 1:
                                    emit_tail(p, i)

                            u.append(step)

                for f in _interleave(units_b):
                    f()

    nc.compile()
    return nc


def _shard_inputs(Q, K, V):
    """Per-core in_maps; core c gets global (b,h) pairs 2c and 2c+1."""
    Q = np.asarray(Q, dtype=np.float32)
    K = np.asarray(K, dtype=np.float32)
    V = np.asarray(V, dtype=np.float32)
    b, t, h, d = Q.shape
    ident = np.eye(128, dtype=np.float32)
    ones16 = np.ones((128, NKB), dtype=np.float32)
    in_maps = []
    for c in range(N_CORES):
        m = {"ident": ident, "ones16": ones16}
        for p in range(PAIRS):
            g = PAIRS * c + p
            bb, hh = divmod(g, h)
            m[f"q{p}"] = np.ascontiguousarray(Q[bb, :, hh, :])
            m[f"k{p}"] = np.ascontiguousarray(K[bb, :, hh, :])
            m[f"v{p}"] = np.ascontiguousarray(V[bb, :, hh, :])
        in_maps.append(m)
    return in_maps


def kernel(Q, K, V, chunk_count, trace=False):
    Q = np.asarray(Q)
    b, t, h, d = Q.shape
    assert (b, t, h, d) == (2, T, 8, D), (b, t, h, d)
    assert T % int(chunk_count) == 0

    if "nc" not in _CACHE:
        _CACHE["nc"] = build_nc()
    nc = _CACHE["nc"]

    in_maps = _shard_inputs(Q, K, V)
    res = run_bass_kernel_spmd(nc, in_maps, core_ids=list(range(N_CORES)), trace=trace)

    out = np.empty((b, t, h, d), dtype=np.float32)
    for c in range(N_CORES):
        for p in range(PAIRS):
            g = PAIRS * c + p
            bb, hh = divmod(g, h)
            out[bb, :, hh, :] = res.results[c][f"o{p}"]
    if trace:
        return out, res
    return out


# revision 51
# speedup vs baseline: 1.0114x; 1.0114x over previous
"""Trainium2 Bass kernel for causal degree-2 polynomial attention.

The reference module is chunked linear attention with kernel weight
(q.k)^2, which is mathematically exact causal polynomial attention:

    out_q = sum_{k<=q} (Q_q.K_k)^2 V_k / (EPS + sum_{k<=q} (Q_q.K_k)^2)

Sharding: 16 (batch, head) pairs across 8 cores -> 2 pairs/core, fully
data-parallel (matches the chunk-local-cumsum hint; no collectives).

Per-core device algorithm (the two pairs are emitted interleaved so
their independent dependency chains fill each other's pipeline stalls):
  - load Q,K,V [2048, 64] fp32 on both HWDGE queues, cast to bf16 on
    DVE in 256-col chunks, PE-transpose Q,K (pair-stacked) into bf16
    [d, t] layouts; Q duplicated onto both partition halves with
    per-block SBUF->SBUF DMAs so D'[k,q] = K Q^T runs as two row-tiled
    K=64 bf16 matmuls per slot with N=512 query blocks.
  - square D' (fp32 PSUM -> bf16 SBUF on ACT, every 5th on DVE via
    cast-copy + multiply), causal-mask diagonal 128-col windows with
    gpsimd affine_select, zero-fill fully masked columns.
  - accumulate [V|1]^T C' into PSUM [65, 512] (bf16 x bf16 -> fp32).
  - PE-transpose back to [q, 65], reciprocal(Z+eps), scale, store each
    query block as soon as it finishes.
"""

import os
import sys

for _p in ("/root/.axon_site", "/root/.axon_site/_ro/trn_rl_repo",
           "/root/.axon_site/_ro/pypackages", "/opt/trn_rl_repo", "/opt/pypackages"):
    if os.path.isdir(_p) and _p not in sys.path:
        sys.path.append(_p)

import numpy as np

import concourse.bacc as bacc
import concourse.mybir as mybir
import concourse.tile as tile
from concourse.bass_utils import run_bass_kernel_spmd

F32 = mybir.dt.float32
BF16 = mybir.dt.bfloat16
EPS = 1e-5

N_CORES = 8
T = 2048          # tokens
D = 64            # head dim
PAIRS = 2         # (b, h) pairs per core
NKB = T // 128    # 16 key blocks of 128
QB = 512          # query block width
NQB = T // QB     # 4 query blocks

_CACHE = {}


def _interleave(lists):
    out = []
    n = max(len(l) for l in lists)
    for j in range(n):
        for l in lists:
            if j < len(l):
                out.append(l[j])
    return out


def build_nc(mmdt=BF16, dve_every=5):
    nc = bacc.Bacc("TRN2", target_bir_lowering=False, debug=False)
    MMDT = mmdt

    ins = {}
    outs = {}
    for p in range(PAIRS):
        for nm in ("q", "k", "v"):
            ins[f"{nm}{p}"] = nc.dram_tensor(f"{nm}{p}", [T, D], F32, kind="ExternalInput").ap()
        outs[p] = nc.dram_tensor(f"o{p}", [T, D], F32, kind="ExternalOutput").ap()
    ident = nc.dram_tensor("ident", [128, 128], F32, kind="ExternalInput").ap()
    ones16 = nc.dram_tensor("ones16", [128, NKB], F32, kind="ExternalInput").ap()

    with tile.TileContext(nc) as tc:
        with (
            tc.tile_pool(name="const", bufs=1) as cpool,
            tc.tile_pool(name="persist", bufs=1) as perpool,
            tc.tile_pool(name="cprime", bufs=8) as cppool,
            tc.tile_pool(name="small", bufs=6) as smpool,
        ):
            ident_sb = cpool.tile([128, 128], F32, name="ident_sb")
            nc.sync.dma_start(ident_sb[:], ident[:])
            identm_sb = cpool.tile([128, 128], MMDT, name="identm_sb")
            nc.vector.tensor_copy(identm_sb[:], ident_sb[:])

            # ---- persistent per-pair SBUF tensors ----
            qT2 = []   # [128, 2048] mmdt, both halves hold Q^T (duplicated)
            kT2 = []   # [128, 1024] mmdt, top: even key blocks, bottom: odd
            v1 = []    # [128, 16*65] mmdt, col 64 of each 65-group is 1.0
            obuf = []  # [128, 16*64] f32 staged output
            for p in range(PAIRS):
                qT2.append(perpool.tile([128, T], MMDT, name=f"qT2_{p}"))
                kT2.append(perpool.tile([128, T // 2], MMDT, name=f"kT2_{p}"))
                v1.append(perpool.tile([128, NKB * 65], MMDT, name=f"v1_{p}"))
                obuf.append(perpool.tile([128, NKB * 64], F32, name=f"obuf_{p}"))

            # ---- stage A: loads, bf16 casts, PE transposes ----
            with (
                tc.tile_pool(name="stage", bufs=2) as stpool,
                tc.tile_pool(name="pstA", bufs=3, space="PSUM") as pstA,
            ):
                nat = {}
                units_a = [[] for _ in range(PAIRS)]
                for p in range(PAIRS):
                    u = units_a[p]

                    def prep(p=p):
                        qn = stpool.tile([128, T // 2], F32, name=f"qn_{p}", tag=f"natq{p}")
                        kn = stpool.tile([128, T // 2], F32, name=f"kn_{p}", tag=f"natk{p}")
                        qnb = stpool.tile([128, T // 2], MMDT, name=f"qnb_{p}", tag=f"natqb{p}")
                        knb = stpool.tile([128, T // 2], MMDT, name=f"knb_{p}", tag=f"natkb{p}")
                        nat[p] = (qnb, knb, qn, kn)

                    def loadchunk(p=p, c=0):
                        # 128KB load chunks pipelined against cast + transpose
                        qnb, knb, qn, kn = nat[p]
                        dmae = nc.sync if p == 0 else nc.scalar
                        cp = (lambda d, s: nc.vector.tensor_copy(d, s)) if p == 0 \
                            else (lambda d, s: nc.scalar.copy(d, s))
                        rows = slice(512 * c, 512 * (c + 1))
                        ck = slice(256 * c, 256 * (c + 1))
                        for dst, dstb, nm in ((qn, qnb, "q"), (kn, knb, "k")):
                            dmae.dma_start(
                                dst[:, ck].rearrange("p (n d) -> p n d", n=4),
                                ins[f"{nm}{p}"][rows, :].rearrange("(n p) d -> p n d", p=128),
                            )
                            cp(dstb[:, ck], dst[:, ck])

                    def loads_v(p=p):
                        vn = stpool.tile([128, T // 2], F32, name=f"vn_{p}", tag=f"natv{p}")
                        on = stpool.tile([128, NKB], F32, name=f"on_{p}", tag="ones")
                        dmae = nc.sync if p == 0 else nc.scalar
                        dmae.dma_start(
                            vn[:].rearrange("p (n d) -> p n d", n=NKB),
                            ins[f"v{p}"].rearrange("(n p) d -> p n d", p=128),
                        )
                        dmae.dma_start(on[:], ones16[:])
                        cp = (lambda d, s: nc.vector.tensor_copy(d, s)) if p == 0 \
                            else (lambda d, s: nc.scalar.copy(d, s))
                        v1g = v1[p][:].rearrange("p (n c) -> p n c", c=65)
                        cp(v1g[:, :, 64], on[:])
                        cp(v1g[:, :, 0:64], vn[:].rearrange("p (n d) -> p n d", n=NKB))

                    u.append(prep)
                    u.append(lambda p=p: loadchunk(p, 0))
                    u.append(lambda p=p: loadchunk(p, 1))
                    for a in range(NKB // 2):
                        if a == 2:
                            u.append(lambda p=p: loadchunk(p, 2))
                        if a == 4:
                            u.append(lambda p=p: loadchunk(p, 3))
                        if a == 6:
                            u.append(loads_v)

                        def transp(p=p, a=a):
                            qnb, knb = nat[p][0], nat[p][1]
                            # two adjacent token blocks transposed at once:
                            # rows 0:64 = block 2a ^T, 64:128 = block 2a+1 ^T
                            tq = pstA.tile([128, 128], MMDT, name="tq", tag="tp")
                            nc.tensor.transpose(tq[:], qnb[:, a * 128:(a + 1) * 128], identm_sb[:])
                            tk = pstA.tile([128, 128], MMDT, name="tk", tag="tp")
                            nc.tensor.transpose(tk[:], knb[:, a * 128:(a + 1) * 128], identm_sb[:])
                            cp = (lambda d, s: nc.vector.tensor_copy(d, s)) if p == 0 \
                                else (lambda d, s: nc.scalar.copy(d, s))
                            cp(kT2[p][:, a * 128:(a + 1) * 128], tk[:])
                            c0 = slice(256 * a, 256 * a + 128)
                            c1 = slice(256 * a + 128, 256 * a + 256)
                            cp(qT2[p][0:64, c0], tq[0:64, :])
                            cp(qT2[p][0:64, c1], tq[64:128, :])
                            if a % 2 == 1:
                                # duplicate finished q-block cols onto bottom half
                                i = a // 2
                                cols = slice(i * QB, (i + 1) * QB)
                                nc.sync.dma_start(qT2[p][64:128, cols], qT2[p][0:64, cols])

                        u.append(transp)

                for f in _interleave(units_a):
                    f()

            # ---- stage B: main blocked attention ----
            with (
                tc.tile_pool(name="psd", bufs=6, space="PSUM") as psd,
                tc.tile_pool(name="psyz", bufs=2, space="PSUM") as psyz,
            ):
                tails = {}
                state = {"n": 0}

                def square(dst, srcap, cols):
                    # ACT squares straight from PSUM; every dve_every-th
                    # block goes to DVE as cast-copy + SBUF multiply.
                    state["n"] += 1
                    if state["n"] % dve_every == 0:
                        tmp = cppool.tile([128, cols], MMDT, name="sqtmp", tag="sqtmp")
                        nc.vector.tensor_copy(tmp[:], srcap)
                        nc.vector.tensor_mul(dst, tmp[:], tmp[:])
                    else:
                        nc.scalar.square(dst, srcap)

                def emit_tail(p, i):
                    """finish query block i of pair p: psumYZ -> output rows."""
                    yz = smpool.tile([65, QB], F32, name="yz", tag="yz")
                    nc.vector.tensor_copy(yz[:], tails[(p, i)][:])
                    for half in range(QB // 128):
                        blk = (QB // 128) * i + half
                        pt = psd.tile([128, 65], F32, name="pt", tag="psd")
                        nc.tensor.transpose(
                            pt[:], yz[:, half * 128:(half + 1) * 128],
                            ident_sb[0:65, 0:65],
                        )
                        zs = smpool.tile([128, 1], F32, name="zs", tag="zs")
                        nc.vector.tensor_scalar_add(zs[:], pt[:, 64:65], EPS)
                        rz = smpool.tile([128, 1], F32, name="rz", tag="rz")
                        nc.vector.reciprocal(rz[:], zs[:])
                        nc.vector.tensor_scalar_mul(
                            obuf[p][:, blk * 64:(blk + 1) * 64], pt[:, 0:64], rz[:]
                        )
                    rows = slice(i * QB, (i + 1) * QB)
                    nc.sync.dma_start(
                        outs[p][rows, :].rearrange("(n p) d -> p n d", p=128),
                        obuf[p][:, i * (QB // 128) * 64:(i + 1) * (QB // 128) * 64]
                            .rearrange("p (n d) -> p n d", d=64),
                    )

                def emit_dpair(p, i, a):
                    """D'[k, q] for key blocks (2a, 2a+1) vs query block i."""
                    qcols = slice(i * QB, (i + 1) * QB)
                    kcols = slice(a * 128, (a + 1) * 128)
                    psA = psd.tile([128, QB], F32, name="psA", tag="psd")
                    psB = psd.tile([128, QB], F32, name="psB", tag="psd")
                    nc.tensor.matmul(
                        psA[:], kT2[p][0:64, kcols], qT2[p][0:64, qcols],
                        start=True, stop=True, tile_position=(0, 0),
                        skip_group_check=True,
                    )
                    nc.tensor.matmul(
                        psB[:], kT2[p][64:128, kcols], qT2[p][64:128, qcols],
                        start=True, stop=True, tile_position=(64, 0),
                        skip_group_check=True,
                    )
                    c2 = cppool.tile([128, 2 * QB], MMDT, name="c2", tag="cp")
                    cE = c2[:, 0:QB]
                    cO = c2[:, QB:2 * QB]
                    for par, (ps, cX) in enumerate(((psA, cE), (psB, cO))):
                        r = 2 * a + par - (QB // 128) * i  # diag sub-block
                        if r < 0:
                            square(cX, ps[:], QB)
                            continue
                        w0 = 128 * r
                        if w0 > 0:
                            nc.gpsimd.memset(cX[:, 0:w0], 0.0)
                        square(cX[:, w0:QB], ps[:, w0:QB], QB - w0)
                        # causal mask on the 128-col diagonal window
                        # (keep where q - k >= 0; iota = f + cm*p + base)
                        nc.gpsimd.affine_select(
                            cX[:, w0:w0 + 128], cX[:, w0:w0 + 128],
                            pattern=[[1, 128]],
                            compare_op=mybir.AluOpType.is_ge, fill=0.0,
                            base=0, channel_multiplier=-1,
                        )
                    return cE, cO

                def emit_cv(p, i, a, cE, cO):
                    yzp = tails[(p, i)]
                    vE = v1[p][:, (2 * a) * 65:(2 * a + 1) * 65]
                    vO = v1[p][:, (2 * a + 1) * 65:(2 * a + 2) * 65]
                    nc.tensor.matmul(
                        yzp[:], vE, cE,
                        start=(a == 0), stop=False, skip_group_check=True,
                    )
                    nc.tensor.matmul(
                        yzp[:], vO, cO,
                        start=False, stop=(a == 2 * i + 1), skip_group_check=True,
                    )

                pend = {}
                units_b = [[] for _ in range(PAIRS)]
                for p in range(PAIRS):
                    u = units_b[p]
                    for i in range(NQB):
                        for a in range(2 * i + 2):
                            def step(p=p, i=i, a=a):
                                if (i, a) == (0, 0):
                                    pend[p] = emit_dpair(p, 0, 0)
                                cur = pend[p]
                                # software pipeline: next D' ahead of this CV
                                if a < 2 * i + 1:
                                    pend[p] = emit_dpair(p, i, a + 1)
                                elif i + 1 < NQB:
                                    pend[p] = emit_dpair(p, i + 1, 0)
                                if a == 0:
                                    tails[(p, i)] = psyz.tile(
                                        [65, QB], F32, name=f"yzp_{p}_{i}", tag="yzp"
                                    )
                                emit_cv(p, i, a, *cur)
                                if a == 2 * i + 1:
                                    emit_tail(p, i)

                            u.append(step)

                for f in _interleave(units_b):
                    f()

    nc.compile()
    return nc


def _shard_inputs(Q, K, V):
    """Per-core in_maps; core c gets global (b,h) pairs 2c and 2c+1."""
    Q = np.asarray(Q, dtype=np.float32)
    K = np.asarray(K, dtype=np.float32)
    V = np.asarray(V, dtype=np.float32)
    b, t, h, d = Q.shape
    ident = np.eye(128, dtype=np.float32)
    ones16 = np.ones((128, NKB), dtype=np.float32)
    in_maps = []
    for c in range(N_CORES):
        m = {"ident": ident, "ones16": ones16}
        for p in range(PAIRS):
            g = PAIRS * c + p
            bb, hh = divmod(g, h)
            m[f"q{p}"] = np.ascontiguousarray(Q[bb, :, hh, :])
            m[f"k{p}"] = np.ascontiguousarray(K[bb, :, hh, :])
            m[f"v{p}"] = np.ascontiguousarray(V[bb, :, hh, :])
        in_maps.append(m)
    return in_maps


def kernel(Q, K, V, chunk_count, trace=False):
    Q = np.asarray(Q)
    b, t, h, d = Q.shape
    assert (b, t, h, d) == (2, T, 8, D), (b, t, h, d)
    assert T % int(chunk_count) == 0

    if "nc" not in _CACHE:
        _CACHE["nc"] = build_nc()
    nc = _CACHE["nc"]

    in_maps = _shard_inputs(Q, K, V)
    res = run_bass_kernel_spmd(nc, in_maps, core_ids=list(range(N_CORES)), trace=trace)

    out = np.empty((b, t, h, d), dtype=np.float32)
    for c in range(N_CORES):
        for p in range(PAIRS):
            g = PAIRS * c + p
            bb, hh = divmod(g, h)
            out[bb, :, hh, :] = res.results[c][f"o{p}"]
    if trace:
        return out, res
    return out
